# revision 4
# baseline (speedup 1.0000x reference)
"""CartesianMACE rank-0 kernel for 8 Trainium2 NeuronCores (Bass/Tile).

Only the rank-0 (scalar) channel chain affects the model output.  Per layer:
  hm   = h0s @ ab_w[l,0].T            ;  A0 = segsum(remb * hm[src], dst)
  msgs = poly(A0)                     ;  h' = cw@h + mw@msgs (per node)
out = pred_w @ colsum(h_final) + pred_b

Structure (vs the v1 baseline, 917us -> ~570us):
 - No startup AllGather: h0 is an input, so every core computes the FULL
   layer-0 hm table locally (no cross-core barrier at startup).  The
   channel mix runs mostly on the otherwise-idle TensorEngine
   ([16x128]^T @ [16x16] matmuls from a host-transposed h0, PSUM slabs
   copied to SBUF), partly on DVE; DVE also precomputes the whole
   remb/sin chain during setup (it depends only on host-fed pos[src]
   per edge slot, not on any gather).
 - Gathers fetch 64B per edge (elem_size=C) from 256B-strided table
   rows via a raw-emitted InstDMAGatherAnt: the %256 elem_size assert
   in bass.dma_gather is a transpose-mode restriction only.  This cut
   the edge-phase DMA traffic 4x (the single biggest win).
 - Gather calls are merged per group-PAIR and chunked to <=15 d-cols
   (~1920 idx, one SWDGE ring's worth), rotating all 4 queues.
 - The gather table uses PARTITION-MAJOR row ids (row = p*GT + gg), so
   table-build DMAs write one contiguous multi-KB run per partition
   instead of one 64B descriptor per row.
 - The inter-layer AllGather of hm1 runs as 2 chunks: everything that
   finishes before the last edge pair rides one early collective
   (hidden under layer-0 gathers); only the last-finishing node batch
   rides the small exposed tail collective.  wtab is double-buffered
   so the layer-1 table build overlaps layer-0 reads.
 - cw/mw node weights stream as fp16; all (j,k) weight layouts are
   row-major so every DVE operand has a contiguous innermost dim; the
   sin range reduction is a fused magic-round (tensor_scalar_add +
   scalar_tensor_tensor) instead of a slow dual-op tensor_scalar.
"""

import sys

import numpy as np

if "/opt/trn_rl_repo" not in sys.path:
    sys.path.insert(0, "/opt/trn_rl_repo")

import concourse.bacc as bacc
import concourse.mybir as mybir
import concourse.tile as tile
from concourse import library_config
from concourse.bass_utils import run_bass_kernel_spmd

NCORES = 8
P = 128
ROW = 64  # gather-table row width in fp32 elems (256B, dma_gather minimum)
NSPLIT = 4  # gather splits per group
NQ = 4  # SWDGE queues
SLAB = 16  # hm0 table groups per PE/copy/widen slab

F32 = mybir.dt.float32
F16 = mybir.dt.float16
I16 = mybir.dt.int16


def _pair_order(G):
    G2 = G // 2
    pairs = [(p, G - 1 - p) for p in range(G2)]
    return pairs[1:4] + [pairs[0]] + pairs[4:]


MAXQ = 15  # max gather d-cols per call


def _chunk_plan(Dg, pairs):
    """Per pair: near-even d-col chunks of <= MAXQ over the pair's merged
    slot range.  Returns [(pair_idx, [(doff, dq), ...])] — one gather call
    per chunk (launches are Q7-cluster-serial, so fewer is better)."""
    plan = []
    for pi, (ga, gb) in enumerate(pairs):
        DP = Dg[ga] + Dg[gb]
        n = -(-DP // MAXQ)
        base = DP // n
        rem = DP - base * n
        chunks = []
        doff = 0
        for i in range(n):
            dq = base + (1 if i < rem else 0)
            chunks.append((doff, dq))
            doff += dq
        plan.append(chunks)
    return plan


def _build_nc(Dg, C, npad):
    Dg = list(Dg)
    G = len(Dg)
    nc = bacc.Bacc(
        "TRN2",
        target_bir_lowering=False,
        debug=False,
        num_devices=NCORES,
        num_swdge_queues=NQ,
    )
    ntab = NCORES * npad
    GT = NCORES * G  # total table groups; table row id = p*GT + gg
    IW = sum(8 * d for d in Dg)
    pairs = _pair_order(G)
    plan = _chunk_plan(Dg, pairs)
    off_pc = []  # idx col offset per (pair, chunk)
    acc_i = 0
    for chunks in plan:
        offs = []
        for doff, dq in chunks:
            offs.append(acc_i)
            acc_i += 8 * dq
        off_pc.append(offs)
    assert acc_i == IW
    off_s = [0] * G  # slot offsets, pair-contiguous
    acc_s = 0
    for ga, gb in pairs:
        off_s[ga] = acc_s
        acc_s += Dg[ga]
        off_s[gb] = acc_s
        acc_s += Dg[gb]
    SD = acc_s
    RW = SD * C

    PEG = 112  # hm0 groups on the TensorEngine; the rest go to DVE
    assert PEG % SLAB == 0
    NSLAB = PEG // SLAB
    DVG = GT - PEG
    GN0 = 8  # DVE hm0 batch width
    assert DVG % GN0 == 0

    # gather calls rotate all 4 SWDGE queues: each queue's ring drains on its
    # own engine group, so the burst pace is per-queue-serial (measured
    # ~6.5us per 1920-row call on one queue regardless of payload size).
    GQUEUES = [0, 1, 2, 3]
    qctr = [0]

    def next_q():
        q = GQUEUES[qctr[0] % len(GQUEUES)]
        qctr[0] += 1
        return q

    # ---- I/O ----
    idx_in = nc.dram_tensor("idx_in", [P, IW], I16, kind="ExternalInput")
    posd_in = nc.dram_tensor("posd_in", [P, G * 3], F32, kind="ExternalInput")
    possrc_in = nc.dram_tensor("possrc_in", [P, SD * 3], F32, kind="ExternalInput")
    h0t_in = nc.dram_tensor("h0t_in", [C, PEG * P], F32, kind="ExternalInput")
    h0tail_in = nc.dram_tensor("h0tail_in", [P, DVG * C], F32, kind="ExternalInput")
    h0_in = nc.dram_tensor("h0_in", [P, G * C], F32, kind="ExternalInput")
    cw0_in = nc.dram_tensor("cw0_in", [G, P, C * C], F16, kind="ExternalInput")
    mw0_in = nc.dram_tensor("mw0_in", [G, P, C * C], F16, kind="ExternalInput")
    cw1_in = nc.dram_tensor("cw1_in", [G, P, C * C], F16, kind="ExternalInput")
    mw1_in = nc.dram_tensor("mw1_in", [G, P, C * C], F16, kind="ExternalInput")
    abw0t_in = nc.dram_tensor("abw0t_in", [C, C], F32, kind="ExternalInput")
    abw0_in = nc.dram_tensor("abw0_in", [P, C * C], F32, kind="ExternalInput")
    abw1_in = nc.dram_tensor("abw1_in", [P, C * C], F32, kind="ExternalInput")
    wsw0_in = nc.dram_tensor("wsw0_in", [P, 3 * C], F32, kind="ExternalInput")
    wsw1_in = nc.dram_tensor("wsw1_in", [P, 3 * C], F32, kind="ExternalInput")
    cvec_in = nc.dram_tensor("cvec_in", [P, C], F32, kind="ExternalInput")
    predw_in = nc.dram_tensor("predw_in", [P, G * 2], F32, kind="ExternalInput")
    predb_in = nc.dram_tensor("predb_in", [1, 2], F32, kind="ExternalInput")
    ones_in = nc.dram_tensor("ones_in", [P, 1], F32, kind="ExternalInput")
    out_t = nc.dram_tensor("out_part", [1, 2], F32, kind="ExternalOutput")

    groups = [list(range(NCORES))]

    GN = 4  # node-update batch width (groups)
    assert G % GN == 0
    NB = G // GN

    def _batch_ready(bi):
        done = set()
        for pi, (a, b) in enumerate(pairs):
            done.add(a)
            done.add(b)
            if all(g in done for g in range(bi * GN, (bi + 1) * GN)):
                return pi
        return len(pairs) - 1

    b_ready = {bi: _batch_ready(bi) for bi in range(NB)}
    # AG1 in 2 chunks: every batch that finishes before the final pair rides
    # one early collective (hidden under layer-0 edge work); only the
    # last-finishing batches ride the small exposed final collective.
    # (each collective op costs ~17us of CC time regardless of payload)
    last_pi = len(pairs) - 1
    late = [bi for bi in range(NB) if b_ready[bi] >= last_pi]
    early = sorted((bi for bi in range(NB) if bi not in late),
                   key=lambda bi: (b_ready[bi], bi))
    ag_chunks = [c for c in (early, late) if c]
    batch_chunk = {}
    batch_coff = {}
    for k, bs in enumerate(ag_chunks):
        for i, bi in enumerate(bs):
            batch_chunk[bi] = k
            batch_coff[bi] = i
    chunk_ready = {k: max(b_ready[bi] for bi in bs) for k, bs in enumerate(ag_chunks)}

    with tile.TileContext(nc) as tc:
        with (
            tc.tile_pool(name="const", bufs=1) as cpool,
            tc.tile_pool(name="resident", bufs=1) as rpool,
            tc.tile_pool(name="gath", bufs=8) as gpool,
            tc.tile_pool(name="work", bufs=2) as wpool,
            tc.tile_pool(name="sinp", bufs=2) as snpool,
            tc.tile_pool(name="small", bufs=3) as spool,
            tc.tile_pool(name="wstream", bufs=2) as wspool,
            tc.tile_pool(name="hm0", bufs=2) as hpool,
            tc.tile_pool(name="psum", bufs=2, space="PSUM") as ppool,
            tc.tile_pool(name="psumf", bufs=1, space="PSUM") as pfpool,
            tc.tile_pool(name="dram", bufs=1, space="DRAM") as dpool,
        ):
            nc.gpsimd.load_library(library_config.mlp)

            # ---- constants / resident data ----
            cvec_sb = cpool.tile([P, C], F32)
            abw0t_sb = cpool.tile([C, C], F32)
            abw1_sb = cpool.tile([P, C * C], F32)
            wsw0_sb = cpool.tile([P, 3 * C], F32)
            wsw1_sb = cpool.tile([P, 3 * C], F32)
            ones_sb = cpool.tile([P, 1], F32)
            predb_sb = cpool.tile([1, 2], F32)

            idx_sb = rpool.tile([P, IW], I16)
            a0_sb = rpool.tile([P, G * C], F32)
            posd_sb = rpool.tile([P, G * 3], F32)
            possrc_sb = rpool.tile([P, SD * 3], F32)
            h0_sb = rpool.tile([P, G * C], F32)
            predw_sb = rpool.tile([P, G * 2], F32)
            remb_sb = rpool.tile([P, RW], F16)
            h1_sb = rpool.tile([P, G * C], F32)
            acc_sb = rpool.tile([P, 2], F32)

            nc.scalar.dma_start(abw0t_sb[:], abw0t_in[:])
            nc.gpsimd.dma_start(idx_sb[:], idx_in[:])
            nc.sync.dma_start(possrc_sb[:], possrc_in[:])
            nc.sync.dma_start(posd_sb[:], posd_in[:])
            nc.sync.dma_start(h0_sb[:], h0_in[:])
            nc.scalar.dma_start(cvec_sb[:], cvec_in[:])
            nc.scalar.dma_start(abw1_sb[:], abw1_in[:])
            nc.scalar.dma_start(wsw0_sb[:], wsw0_in[:])
            nc.scalar.dma_start(wsw1_sb[:], wsw1_in[:])
            nc.scalar.dma_start(ones_sb[:], ones_in[:])
            nc.scalar.dma_start(predb_sb[:], predb_in[:])
            nc.scalar.dma_start(predw_sb[:], predw_in[:])
            nc.vector.memset(acc_sb[:], 0.0)
            # fold remb's sqrt(2) into the layer-0/1 atomic-basis weights
            nc.vector.tensor_scalar_mul(
                out=abw0t_sb[:], in0=abw0t_sb[:], scalar1=float(np.sqrt(2.0))
            )
            nc.vector.tensor_scalar_mul(
                out=abw1_sb[:], in0=abw1_sb[:], scalar1=float(np.sqrt(2.0))
            )

            # ---- DRAM: wide gather tables (rows: p*GT + gg) + AG1 bounces
            wtab0 = dpool.tile([ntab, ROW], F32)
            wtab1 = dpool.tile([ntab, ROW], F32)
            ag_ins = [
                dpool.tile([len(bs) * GN * P, C], F32, name=f"ag1i_{k}")
                for k, bs in enumerate(ag_chunks)
            ]
            ag_outs = [
                dpool.tile([NCORES * len(bs) * GN * P, C], F32, name=f"ag1o_{k}")
                for k, bs in enumerate(ag_chunks)
            ]

            # ---- layer-0 full hm table, replicated: PE (112 groups) and
            # DVE (48 groups) build it concurrently ----
            widen_engines = [nc.sync, nc.scalar, nc.gpsimd]
            h0tail_sb = rpool.tile([P, DVG * C], F32)
            nc.scalar.dma_start(h0tail_sb[:], h0tail_in[:])
            abw0_sb = cpool.tile([P, C * C], F32)
            nc.scalar.dma_start(abw0_sb[:], abw0_in[:])
            nc.vector.tensor_scalar_mul(
                out=abw0_sb[:], in0=abw0_sb[:], scalar1=float(np.sqrt(2.0))
            )
            for sb in range(NSLAB):
                h0t_t = hpool.tile([C, SLAB * P], F32, name=f"h0t_{sb}", tag="h0t")
                nc.sync.dma_start(
                    h0t_t[:], h0t_in[:, sb * SLAB * P : (sb + 1) * SLAB * P]
                )
                ps = ppool.tile([P, SLAB * C], F32, name=f"ps_{sb}", tag="ps")
                for gl in range(SLAB):
                    nc.tensor.matmul(
                        out=ps[:, gl * C : (gl + 1) * C],
                        lhsT=h0t_t[:, gl * P : (gl + 1) * P],
                        rhs=abw0t_sb[:],
                        start=True,
                        stop=True,
                    )
                hsl = hpool.tile([P, SLAB * ROW], F32, name=f"hsl_{sb}", tag="hsl")
                nc.vector.tensor_copy(
                    out=hsl[:].rearrange("p (g w) -> p g w", w=ROW)[:, :, 0:C],
                    in_=ps[:].rearrange("p (g c) -> p g c", c=C),
                )
                weng = widen_engines[sb % 3]
                weng.dma_start(
                    wtab0[:, :]
                    .rearrange("(p q) w -> p q w", q=GT)[
                        :, sb * SLAB : (sb + 1) * SLAB, :
                    ],
                    hsl[:].rearrange("p (g w) -> p g w", w=ROW),
                )

            for bb in range(DVG // GN0):
                gg0 = PEG + bb * GN0
                h8 = h0tail_sb[:, bb * GN0 * C : (bb + 1) * GN0 * C]
                t = hpool.tile([P, GN0 * C * C], F32, name=f"hdT_{bb}", tag="hdT")
                nc.vector.tensor_mul(
                    out=t[:].rearrange("p (g j k) -> p g j k", j=C, k=C),
                    in0=abw0_sb[:]
                    .rearrange("p (j k) -> p j k", k=C)
                    .to_broadcast([P, C, C, GN0])
                    .rearrange("p j k g -> p g j k"),
                    in1=h8.rearrange("p (g k) -> p g k", k=C)
                    .to_broadcast([P, GN0, C, C])
                    .rearrange("p g k j -> p g j k"),
                )
                hd = hpool.tile([P, GN0 * ROW], F32, name=f"hd_{bb}", tag="hd")
                nc.vector.reduce_sum(
                    out=hd[:].rearrange("p (g w) -> p g w", w=ROW)[:, :, 0:C],
                    in_=t[:].rearrange("p (g j k) -> p g j k", j=C, k=C),
                    axis=mybir.AxisListType.X,
                )
                weng = widen_engines[bb % 3]
                weng.dma_start(
                    wtab0[:, :]
                    .rearrange("(p q) w -> p q w", q=GT)[:, gg0 : gg0 + GN0, :],
                    hd[:].rearrange("p (g w) -> p g w", w=ROW),
                )

            # ---- remb/sin chain for every pair (no gather dependency) ----
            DPmax = max(Dg[ga] + Dg[gb] for ga, gb in pairs)

            def remb_pair(ga, gb):
                Da, Db = Dg[ga], Dg[gb]
                DP = Da + Db
                remb_g = remb_sb[:, off_s[ga] * C : (off_s[ga] + DP) * C]
                ps_ = possrc_sb[:, off_s[ga] * 3 : (off_s[ga] + DP) * 3]
                rel_t = spool.tile([P, DPmax * 3], F32, name=f"rel_{ga}", tag="rel")
                rel = rel_t[:, 0 : DP * 3]
                for gg, doff in ((ga, 0), (gb, Da)):
                    D = Dg[gg]
                    nc.vector.tensor_sub(
                        out=rel_t[:, doff * 3 : (doff + D) * 3].rearrange(
                            "p (d x) -> p d x", x=3
                        ),
                        in0=ps_.rearrange("p (d x) -> p d x", x=3)[
                            :, doff : doff + D, :
                        ],
                        in1=posd_sb[:, gg * 3 : (gg + 1) * 3]
                        .to_broadcast([P, 3, D])
                        .rearrange("p x d -> p d x"),
                    )
                nc.vector.tensor_mul(out=rel, in0=rel, in1=rel)
                dd_t = spool.tile([P, DPmax], F32, name=f"dd_{ga}", tag="dd")
                dd = dd_t[:, 0:DP]
                nc.vector.reduce_sum(
                    out=dd,
                    in_=rel.rearrange("p (d x) -> p d x", x=3),
                    axis=mybir.AxisListType.X,
                )
                dv_t = spool.tile([P, DPmax], F32, name=f"dv_{ga}", tag="dv")
                dv = dv_t[:, 0:DP]
                nc.scalar.sqrt(dv, dd)
                inv_t = spool.tile([P, DPmax], F32, name=f"inv_{ga}", tag="inv")
                inv = inv_t[:, 0:DP]
                nc.vector.reciprocal(inv, dv)
                sin_t = snpool.tile([P, DPmax * C], F32, name=f"sinin_{ga}", tag="sin")
                sinin = sin_t[:, 0 : DP * C]
                nc.vector.tensor_mul(
                    out=sinin.rearrange("p (d c) -> p d c", c=C),
                    in0=dv.to_broadcast([P, DP, C]),
                    in1=cvec_sb[:].to_broadcast([P, C, DP]).rearrange("p c d -> p d c"),
                )
                MAGIC = 12582912.0  # 1.5 * 2**23
                kr_t = snpool.tile([P, DPmax * C], F32, name=f"kr_{ga}", tag="kr")
                kr = kr_t[:, 0 : DP * C]
                # kr = round(sinin) via fp32 magic add; y = round - sinin =
                # -frac (fused); sin(2*pi*frac) = Sin(-2*pi * y)
                nc.vector.tensor_scalar_add(out=kr, in0=sinin, scalar1=MAGIC)
                nc.vector.scalar_tensor_tensor(
                    out=kr,
                    in0=kr,
                    scalar=MAGIC,
                    in1=sinin,
                    op0=mybir.AluOpType.subtract,
                    op1=mybir.AluOpType.subtract,
                )
                sino = sinin
                nc.scalar.activation(
                    sino,
                    kr,
                    mybir.ActivationFunctionType.Sin,
                    scale=float(-2.0 * np.pi),
                )
                nc.vector.tensor_mul(
                    out=remb_g.rearrange("p (d c) -> p d c", c=C),
                    in0=sino.rearrange("p (d c) -> p d c", c=C),
                    in1=inv.to_broadcast([P, DP, C]),
                )

            for ga, gb in pairs:
                remb_pair(ga, gb)

            # ---- edge processing (both layers) ----
            def gather_small(out_ap, in_ap, idxs_ap, num_idxs, queue_num):
                """InstDMAGatherAnt with a 64B payload per index (the %256
                elem_size assert in bass.dma_gather is a transpose-mode
                restriction; the non-transpose Q7 generator handles any
                packet length — only the row STRIDE must be 256B-aligned)."""
                eng = nc.gpsimd
                _in = eng.lower_ap_dma(in_ap, for_custom_bir_dma=True)
                _idx = eng.lower_ap(idxs_ap)
                _out = eng.lower_ap(out_ap)
                return eng.add_instruction(
                    mybir.InstDMAGatherAnt(
                        name=eng.bass.get_next_instruction_name(),
                        ins=[*_in, _idx, eng.lower_val_access(eng.to_reg(num_idxs))],
                        outs=[_out],
                        transpose=False,
                        num_idxs=num_idxs,
                        elem_size=C,
                        stride_bytes_256=(ROW * 4) // 256,
                        gen_mode=0,
                        single_packet=False,
                        queue_num=queue_num,
                    )
                )

            def edge_layer_pair(pi, ga, gb, wtab):
                Da, Db = Dg[ga], Dg[gb]
                DP = Da + Db
                remb_g = remb_sb[:, off_s[ga] * C : (off_s[ga] + DP) * C]
                gt = gpool.tile([P, DPmax * C], F32, name=f"gt_{ga}", tag="gt")
                gv3 = gt[:].rearrange("p (d e) -> p d e", e=C)
                for ci, (doff, dq) in enumerate(plan[pi]):
                    io = off_pc[pi][ci]
                    gather_small(
                        out_ap=gv3[:, doff : doff + dq, :],
                        in_ap=wtab[:, 0:C],
                        idxs_ap=idx_sb[:, io : io + 8 * dq],
                        num_idxs=P * dq,
                        queue_num=next_q(),
                    )
                he = gv3[:, 0:DP, :]
                nc.vector.tensor_mul(
                    out=he,
                    in0=remb_g.rearrange("p (d c) -> p d c", c=C),
                    in1=he,
                )
                for gg, doff in ((ga, 0), (gb, Da)):
                    D = Dg[gg]
                    nc.vector.reduce_sum(
                        out=a0_sb[:, gg * C : (gg + 1) * C],
                        in_=gt[:].rearrange("p (d e) -> p e d", e=C)[
                            :, :, doff : doff + D
                        ],
                        axis=mybir.AxisListType.X,
                    )

            def wsw_b(wsw_sb, k):
                return (
                    wsw_sb[:, k * C : (k + 1) * C]
                    .to_broadcast([P, C, GN])
                    .rearrange("p c g -> p g c")
                )

            def node_update_batch(b, wsw_sb, cw_in_t, mw_in_t, h_src_sb, h_dst_ap):
                s = a0_sb[:, b * C : (b + GN) * C]
                sv = s.rearrange("p (g c) -> p g c", c=C)
                tp = spool.tile([P, GN * C], F32, name=f"tp_{b}", tag="tp")
                tv = tp[:].rearrange("p (g c) -> p g c", c=C)
                nc.vector.tensor_mul(out=tv, in0=sv, in1=wsw_b(wsw_sb, 2))
                nc.vector.tensor_add(out=tv, in0=tv, in1=wsw_b(wsw_sb, 1))
                nc.vector.tensor_mul(out=tp[:], in0=tp[:], in1=s)
                nc.vector.tensor_add(out=tv, in0=tv, in1=wsw_b(wsw_sb, 0))
                msgs = spool.tile([P, GN * C], F32, name=f"msgs_{b}", tag="msgs")
                nc.vector.tensor_mul(out=msgs[:], in0=tp[:], in1=s)

                cw_t = wspool.tile([P, GN * C * C], F16, name=f"cw_{b}", tag="cw")
                mw_t = wspool.tile([P, GN * C * C], F16, name=f"mw_{b}", tag="mw")
                nc.scalar.dma_start(
                    cw_t[:].rearrange("p (g w) -> p g w", g=GN),
                    cw_in_t[:].rearrange("g p w -> p g w")[:, b : b + GN, :],
                )
                nc.scalar.dma_start(
                    mw_t[:].rearrange("p (g w) -> p g w", g=GN),
                    mw_in_t[:].rearrange("g p w -> p g w")[:, b : b + GN, :],
                )
                h4 = h_src_sb[:, b * C : (b + GN) * C]
                t1 = wpool.tile([P, GN * C * C], F32, name=f"t1_{b}", tag="t1")
                nc.vector.tensor_mul(
                    out=t1[:].rearrange("p (g j k) -> p g j k", j=C, k=C),
                    in0=cw_t[:].rearrange("p (g j k) -> p g j k", j=C, k=C),
                    in1=h4.rearrange("p (g k) -> p g k", k=C)
                    .to_broadcast([P, GN, C, C])
                    .rearrange("p g k j -> p g j k"),
                )
                t2 = wpool.tile([P, GN * C * C], F32, name=f"t2_{b}", tag="t2")
                nc.vector.tensor_mul(
                    out=t2[:].rearrange("p (g j k) -> p g j k", j=C, k=C),
                    in0=mw_t[:].rearrange("p (g j k) -> p g j k", j=C, k=C),
                    in1=msgs[:].rearrange("p (g k) -> p g k", k=C)
                    .to_broadcast([P, GN, C, C])
                    .rearrange("p g k j -> p g j k"),
                )
                nc.vector.tensor_add(out=t1[:], in0=t1[:], in1=t2[:])
                nc.vector.reduce_sum(
                    out=h_dst_ap,
                    in_=t1[:].rearrange("p (g j k) -> p g j k", j=C, k=C),
                    axis=mybir.AxisListType.X,
                )

            def hm1_batch(b):
                """hm1 for groups [b, b+GN) -> 256B-padded AG1 chunk rows."""
                h4 = h1_sb[:, b * C : (b + GN) * C]
                t = wpool.tile([P, GN * C * C], F32, name=f"hmT_{b}", tag="t1")
                nc.vector.tensor_mul(
                    out=t[:].rearrange("p (g j k) -> p g j k", j=C, k=C),
                    in0=abw1_sb[:]
                    .rearrange("p (j k) -> p j k", k=C)
                    .to_broadcast([P, C, C, GN])
                    .rearrange("p j k g -> p g j k"),
                    in1=h4.rearrange("p (g k) -> p g k", k=C)
                    .to_broadcast([P, GN, C, C])
                    .rearrange("p g k j -> p g j k"),
                )
                hm = spool.tile([P, GN * C], F32, name=f"hm_{b}", tag="hm")
                nc.vector.reduce_sum(
                    out=hm[:],
                    in_=t[:].rearrange("p (g j k) -> p g j k", j=C, k=C),
                    axis=mybir.AxisListType.X,
                )
                bi = b // GN
                k = batch_chunk[bi]
                off = batch_coff[bi]
                qn = len(ag_chunks[k]) * GN
                dst = ag_ins[k][:].rearrange("(p q) c -> p q c", q=qn)[
                    :, off * GN : (off + 1) * GN, :
                ]
                nc.sync.dma_start(dst, hm[:].rearrange("p (g c) -> p g c", c=C))

            def ag1_chunk(k):
                nc.gpsimd.collective_compute(
                    "AllGather",
                    mybir.AluOpType.bypass,
                    replica_groups=groups,
                    ins=[ag_ins[k][:].opt()],
                    outs=[ag_outs[k][:].opt()],
                )

            def widen1_chunk(k, engines):
                """ag_out rows (n, p, q) compact -> wtab1 rows p*GT + n*G + g
                (64B payload per 256B-strided row), per batch per core."""
                bs = ag_chunks[k]
                qn = len(bs) * GN
                src = ag_outs[k][:].rearrange("(n p q) c -> n p q c", n=NCORES, q=qn)
                dstv = wtab1[:, 0:C].rearrange("(p n g) w -> n p g w", n=NCORES, g=G)
                e = 0
                for i, bi in enumerate(bs):
                    g0 = bi * GN
                    for n in range(NCORES):
                        engines[e % len(engines)].dma_start(
                            dstv[n, :, g0 : g0 + GN, :],
                            src[n, :, i * GN : (i + 1) * GN, :],
                        )
                        e += 1

            def batches_ready_after(pi):
                done = set()
                for q in range(pi + 1):
                    done.add(pairs[q][0])
                    done.add(pairs[q][1])
                return [
                    b for b in range(0, G, GN)
                    if all(g in done for g in range(b, b + GN))
                ]

            # ---- layer 0 ----
            CC_LAG = 0
            emitted = set()
            emitted_cc = set()
            emitted_w = set()
            for pi, (ga, gb) in enumerate(pairs):
                edge_layer_pair(pi, ga, gb, wtab0)
                for b in batches_ready_after(pi):
                    if b in emitted:
                        continue
                    emitted.add(b)
                    node_update_batch(
                        b, wsw0_sb, cw0_in, mw0_in, h0_sb,
                        h1_sb[:, b * C : (b + GN) * C],
                    )
                    hm1_batch(b)
                for k, rp in chunk_ready.items():
                    if k not in emitted_cc and pi >= rp + CC_LAG:
                        emitted_cc.add(k)
                        ag1_chunk(k)
                    if k in emitted_cc and k not in emitted_w and pi >= rp + CC_LAG + 1:
                        emitted_w.add(k)
                        widen1_chunk(k, [nc.sync, nc.scalar])
            for k in range(len(ag_chunks)):
                if k not in emitted_cc:
                    ag1_chunk(k)
            for k in range(len(ag_chunks)):
                if k not in emitted_w:
                    # at the layer boundary the Pool queue is free too
                    widen1_chunk(k, [nc.sync, nc.scalar, nc.gpsimd])

            # ---- layer 1 + head ----
            def head_batch(b):
                hf = spool.tile([P, GN * C], F32, name=f"hf_{b}", tag="hf")
                node_update_batch(b, wsw1_sb, cw1_in, mw1_in, h1_sb, hf[:])
                rs = spool.tile([P, GN], F32, name=f"rs_{b}", tag="rs")
                nc.vector.reduce_sum(
                    out=rs[:],
                    in_=hf[:].rearrange("p (g c) -> p g c", c=C),
                    axis=mybir.AxisListType.X,
                )
                ctr = spool.tile([P, GN * 2], F32, name=f"ctr_{b}", tag="ctr")
                nc.vector.tensor_mul(
                    out=ctr[:].rearrange("p (g t) -> p g t", t=2),
                    in0=predw_sb[:, b * 2 : (b + GN) * 2].rearrange(
                        "p (g t) -> p g t", t=2
                    ),
                    in1=rs[:].to_broadcast([P, GN, 2]),
                )
                cts = spool.tile([P, 2], F32, name=f"cts_{b}", tag="cts")
                nc.vector.reduce_sum(
                    out=cts[:],
                    in_=ctr[:].rearrange("p (g t) -> p t g", t=2),
                    axis=mybir.AxisListType.X,
                )
                nc.vector.tensor_add(out=acc_sb[:], in0=acc_sb[:], in1=cts[:])

            emitted = set()
            for pi, (ga, gb) in enumerate(pairs):
                edge_layer_pair(pi, ga, gb, wtab1)
                for b in batches_ready_after(pi):
                    if b in emitted:
                        continue
                    emitted.add(b)
                    head_batch(b)

            fin_ps = pfpool.tile([1, 2], F32)
            nc.tensor.matmul(
                out=fin_ps[:], lhsT=ones_sb[:], rhs=acc_sb[:], start=True, stop=True
            )
            res = spool.tile([1, 2], F32)
            nc.vector.tensor_add(out=res[:], in0=fin_ps[:], in1=predb_sb[:])
            nc.sync.dma_start(out_t[:], res[:])

    nc.compile()
    return nc


def _wrap_idx16(v):
    """[128, D] int array -> dma_gather idx tile [128, 8*D] int16."""
    p128, d_pad = v.shape
    blk = v.T.reshape(d_pad, 8, 16).transpose(2, 0, 1).reshape(16, 8 * d_pad)
    return np.tile(blk, (8, 1)).astype(np.int16)


def _prep_inputs(pos, h0, ab_w, ws_w, cw, mw, pred_w, pred_b, edge_index):
    """Host-side sharding: degree-sort nodes per core, bucket edges by dst
    into per-group padded degree slots, slice per-node weights.  Data
    movement / index arithmetic only — all model arithmetic runs on
    device."""
    pos = np.asarray(pos, np.float32)
    h0 = np.asarray(h0, np.float32)
    ab_w = np.asarray(ab_w, np.float32)
    ws_w = np.asarray(ws_w, np.float32)
    cw = np.asarray(cw, np.float32)
    mw = np.asarray(mw, np.float32)
    pred_w = np.asarray(pred_w, np.float32)
    pred_b = np.asarray(pred_b, np.float32)
    ei = np.asarray(edge_index)
    src = ei[0].astype(np.int64)
    dst = ei[1].astype(np.int64)

    n_nodes = pos.shape[0]
    c_ch = h0.shape[1]
    assert n_nodes % NCORES == 0
    nc_nodes = n_nodes // NCORES
    g_groups = -(-nc_nodes // P)
    npad = g_groups * P
    GT = NCORES * g_groups
    assert P * GT <= 32767, "int16 gather indices"

    deg_all = np.bincount(dst, minlength=n_nodes)

    perms = []
    inv_slot = np.zeros(n_nodes, np.int64)
    for core in range(NCORES):
        lo = core * nc_nodes
        order = np.argsort(-deg_all[lo : lo + nc_nodes], kind="stable")
        perm = np.concatenate([order, np.arange(nc_nodes, npad)])
        perms.append(perm)
        inv_slot[lo + order] = np.arange(nc_nodes)

    deg_sorted = np.zeros((NCORES, npad), np.int64)
    for core in range(NCORES):
        lo = core * nc_nodes
        deg_sorted[core, :nc_nodes] = deg_all[lo + perms[core][:nc_nodes]]
    Dg = []
    for g in range(g_groups):
        dmax = int(deg_sorted[:, g * P : (g + 1) * P].max())
        Dg.append(max(1, dmax))
    Dg = tuple(Dg)

    pairs = _pair_order(g_groups)
    off_s = [0] * g_groups
    acc_s = 0
    for ga, gb in pairs:
        off_s[ga] = acc_s
        acc_s += Dg[ga]
        off_s[gb] = acc_s
        acc_s += Dg[gb]
    SD = acc_s

    def padded_row(node):
        # table row (partition-major): p*GT + core*G + g
        core = node // nc_nodes
        s = inv_slot[node]
        return (s % P) * GT + core * g_groups + s // P

    # full-table h0 in (gg, p) order; transposed head for the PE, row-major
    # tail for DVE
    PEG = 112
    h0tab = np.zeros((GT, P, c_ch), np.float32)
    for core in range(NCORES):
        lo = core * nc_nodes
        perm = perms[core]
        real = perm < nc_nodes
        hvals = np.zeros((npad, c_ch), np.float32)
        hvals[real] = h0[lo + perm[real], :, 0]
        h0tab[core * g_groups : (core + 1) * g_groups] = hvals.reshape(
            g_groups, P, c_ch
        )
    h0t = (
        h0tab[:PEG].reshape(PEG * P, c_ch).T.copy()
    )  # [C, PEG*P], col = gg*128+p
    h0tail = (
        h0tab[PEG:].transpose(1, 0, 2).reshape(P, (GT - PEG) * c_ch).copy()
    )  # [P, (gg-PEG)*C]

    in_maps = []
    for core in range(NCORES):
        lo, hi = core * nc_nodes, (core + 1) * nc_nodes
        sel = (dst >= lo) & (dst < hi)
        d_loc = inv_slot[dst[sel]]
        s_glb = src[sel]
        order = np.argsort(d_loc, kind="stable")
        d_sort = d_loc[order]
        s_sort = s_glb[order]
        starts = np.searchsorted(d_sort, np.arange(nc_nodes))
        rank = np.arange(len(d_sort)) - starts[d_sort]

        perm = perms[core]
        real = perm < nc_nodes

        posd = np.zeros((npad, 3), np.float32)
        posd[real] = pos[lo + perm[real]]
        posd[~real, 0] = 1e4 + core * npad + np.arange(
            npad - nc_nodes, dtype=np.float32
        )

        # dummy: a PAD row of the next core (its h0t cols are 0)
        sd = nc_nodes  # first pad slot
        dummy = (sd % P) * GT + ((core + 1) % NCORES) * g_groups + sd // P
        idxg = {}
        possrc = np.zeros((P, SD, 3), np.float32)
        for g in range(g_groups):
            D = Dg[g]
            idx = np.full((P, D), dummy, np.int64)
            srcm = np.full((P, D), -1, np.int64)
            in_g = (d_sort >= g * P) & (d_sort < (g + 1) * P)
            rg = d_sort[in_g] - g * P
            rk = rank[in_g]
            idx[rg, rk] = padded_row(s_sort[in_g])
            srcm[rg, rk] = s_sort[in_g]
            idxg[g] = idx
            ps = np.empty((P, D, 3), np.float32)
            valid = srcm >= 0
            ps[valid] = pos[srcm[valid]]
            padmask = ~valid
            pd, _ = np.nonzero(padmask)
            base = posd[g * P : (g + 1) * P]
            ps[padmask] = base[pd] + np.array([1.7, 0.0, 0.0], np.float32)
            possrc[:, off_s[g] : off_s[g] + D, :] = ps
        # pair-major idx layout, wrapped per merged-pair chunk
        plan = _chunk_plan(Dg, pairs)
        idx_cols = []
        for pi, (ga, gb) in enumerate(pairs):
            idxp = np.concatenate([idxg[ga], idxg[gb]], axis=1)
            for doff, dq in plan[pi]:
                idx_cols.append(_wrap_idx16(idxp[:, doff : doff + dq]))
        idx_w = np.concatenate(idx_cols, axis=1)

        h0loc = np.zeros((npad, c_ch), np.float32)
        h0loc[real] = h0[lo + perm[real], :, 0]

        def node_w(warr, layer):
            wloc = np.zeros((npad, c_ch * c_ch), np.float16)
            wloc[real] = (
                warr[layer, 0, lo + perm[real]]
                .reshape(-1, c_ch * c_ch)
                .astype(np.float16)
            )
            return wloc.reshape(g_groups, P, c_ch * c_ch)

        predw = np.zeros((npad, 2), np.float32)
        predw[real] = pred_w[:, lo + perm[real]].T

        rep = lambda v: np.broadcast_to(
            np.asarray(v, np.float32).reshape(1, -1), (P, np.asarray(v).size)
        ).copy()

        def part_major(a):
            K = a.shape[1]
            return (
                a.reshape(g_groups, P, K).transpose(1, 0, 2).reshape(P, g_groups * K)
            )

        in_maps.append(
            {
                "idx_in": idx_w,
                "posd_in": part_major(posd),
                "possrc_in": possrc.reshape(P, SD * 3),
                "h0t_in": h0t,
                "h0tail_in": h0tail,
                "h0_in": part_major(h0loc),
                "cw0_in": node_w(cw, 0),
                "mw0_in": node_w(mw, 0),
                "cw1_in": node_w(cw, 1),
                "mw1_in": node_w(mw, 1),
                "abw0t_in": ab_w[0, 0].T.copy(),
                "abw0_in": rep(ab_w[0, 0].ravel()),
                "abw1_in": rep(ab_w[1, 0].ravel()),
                "wsw0_in": rep(ws_w[0, 0].ravel()),
                "wsw1_in": rep(ws_w[1, 0].ravel()),
                "cvec_in": rep(np.arange(1, c_ch + 1, dtype=np.float32) / 2.0),
                "predw_in": part_major(predw),
                "predb_in": (pred_b if core == 0 else np.zeros(2)).reshape(1, 2)
                .astype(np.float32),
                "ones_in": np.ones((P, 1), np.float32),
            }
        )
    meta = dict(Dg=Dg, C=c_ch, npad=npad)
    return in_maps, meta


_NC_CACHE = {}


def kernel(**inputs) -> np.ndarray:
    in_maps, meta = _prep_inputs(**inputs)
    key = (meta["Dg"], meta["C"], meta["npad"])
    if key not in _NC_CACHE:
        _NC_CACHE[key] = _build_nc(**meta)
    nc = _NC_CACHE[key]
    res = run_bass_kernel_spmd(nc, in_maps, core_ids=list(range(NCORES)))
    parts = [r["out_part"] for r in res.results]
    return np.sum(parts, axis=0).astype(np.float32)


# revision 5
# speedup vs baseline: 1.0013x; 1.0013x over previous
"""CartesianMACE rank-0 kernel for 8 Trainium2 NeuronCores (Bass/Tile), v3.

Only the rank-0 (scalar) channel chain affects the model output.  Per layer:
  hm   = h0s @ ab_w[l,0].T            ;  A0 = segsum(remb * hm[src], dst)
  msgs = poly(A0)                     ;  h' = cw@h + mw@msgs (per node)
out = pred_w @ colsum(h_final) + pred_b

v3 structure (vs the v1 baseline):
 - No startup AllGather: h0 is an input, so every core computes the FULL
   layer-0 hm table locally.  The channel mix runs on the otherwise-idle
   TensorEngine (160 [16x128]^T @ [16x16] matmuls from a host-transposed
   h0), PSUM slabs are copied to SBUF by the Scalar engine, and DVE is
   left free to precompute the whole remb/sin chain (which depends only
   on host-fed pos[src] per edge slot, not on any gather).
 - The gather table uses PARTITION-MAJOR row ids (row = p*GT + gg), so a
   table-build DMA writes one contiguous multi-KB run per partition
   (128 descriptors per slab) instead of one 64B descriptor per row.
 - The inter-layer AllGather of hm1 is split into 3 contiguous-batch
   chunks issued as soon as their node-update batches finish, and its
   payload rows are padded to 256B so the collective writes DIRECTLY
   into the wide gather table (no widening step at all; the unread
   cols 16..63 of each row are garbage).
 - cw/mw node weights stream as fp16; all (j,k) weight layouts are
   row-major so every DVE operand has a contiguous innermost dim.
"""

import sys

import numpy as np

if "/opt/trn_rl_repo" not in sys.path:
    sys.path.insert(0, "/opt/trn_rl_repo")

import concourse.bacc as bacc
import concourse.mybir as mybir
import concourse.tile as tile
from concourse import library_config
from concourse.bass_utils import run_bass_kernel_spmd

NCORES = 8
P = 128
ROW = 64  # gather-table row width in fp32 elems (256B, dma_gather minimum)
NSPLIT = 4  # gather splits per group
NQ = 4  # SWDGE queues
SLAB = 16  # hm0 table groups per PE/copy/widen slab

F32 = mybir.dt.float32
F16 = mybir.dt.float16
I16 = mybir.dt.int16


def _pair_order(G):
    G2 = G // 2
    pairs = [(p, G - 1 - p) for p in range(G2)]
    return pairs[1:4] + [pairs[0]] + pairs[4:]


MAXQ = 15  # max gather d-cols per call


def _chunk_plan(Dg, pairs):
    """Per pair: near-even d-col chunks of <= MAXQ over the pair's merged
    slot range.  Returns [(pair_idx, [(doff, dq), ...])] — one gather call
    per chunk (launches are Q7-cluster-serial, so fewer is better)."""
    plan = []
    for pi, (ga, gb) in enumerate(pairs):
        DP = Dg[ga] + Dg[gb]
        n = -(-DP // MAXQ)
        base = DP // n
        rem = DP - base * n
        chunks = []
        doff = 0
        for i in range(n):
            dq = base + (1 if i < rem else 0)
            chunks.append((doff, dq))
            doff += dq
        plan.append(chunks)
    return plan


def _build_nc(Dg, C, npad):
    Dg = list(Dg)
    G = len(Dg)
    nc = bacc.Bacc(
        "TRN2",
        target_bir_lowering=False,
        debug=False,
        num_devices=NCORES,
        num_swdge_queues=NQ,
    )
    ntab = NCORES * npad
    GT = NCORES * G  # total table groups; table row id = p*GT + gg
    IW = sum(8 * d for d in Dg)
    pairs = _pair_order(G)
    plan = _chunk_plan(Dg, pairs)
    off_pc = []  # idx col offset per (pair, chunk)
    acc_i = 0
    for chunks in plan:
        offs = []
        for doff, dq in chunks:
            offs.append(acc_i)
            acc_i += 8 * dq
        off_pc.append(offs)
    assert acc_i == IW
    off_s = [0] * G  # slot offsets, pair-contiguous
    acc_s = 0
    for ga, gb in pairs:
        off_s[ga] = acc_s
        acc_s += Dg[ga]
        off_s[gb] = acc_s
        acc_s += Dg[gb]
    SD = acc_s
    RW = SD * C

    PEG = 96  # hm0 groups on the TensorEngine; the rest go to DVE
    assert PEG % SLAB == 0
    NSLAB = PEG // SLAB
    DVG = GT - PEG
    GN0 = 8  # DVE hm0 batch width
    assert DVG % GN0 == 0

    # gather calls rotate all 4 SWDGE queues: each queue's ring drains on its
    # own engine group, so the burst pace is per-queue-serial (measured
    # ~6.5us per 1920-row call on one queue regardless of payload size).
    GQUEUES = [0, 1, 2, 3]
    qctr = [0]

    def next_q():
        q = GQUEUES[qctr[0] % len(GQUEUES)]
        qctr[0] += 1
        return q

    # ---- I/O ----
    idx_in = nc.dram_tensor("idx_in", [P, IW], I16, kind="ExternalInput")
    posd_in = nc.dram_tensor("posd_in", [P, G * 3], F32, kind="ExternalInput")
    possrc_in = nc.dram_tensor("possrc_in", [P, SD * 3], F32, kind="ExternalInput")
    h0t_in = nc.dram_tensor("h0t_in", [C, PEG * P], F32, kind="ExternalInput")
    h0tail_in = nc.dram_tensor("h0tail_in", [P, DVG * C], F32, kind="ExternalInput")
    h0_in = nc.dram_tensor("h0_in", [P, G * C], F32, kind="ExternalInput")
    cw0_in = nc.dram_tensor("cw0_in", [G, P, C * C], F16, kind="ExternalInput")
    mw0_in = nc.dram_tensor("mw0_in", [G, P, C * C], F16, kind="ExternalInput")
    cw1_in = nc.dram_tensor("cw1_in", [G, P, C * C], F16, kind="ExternalInput")
    mw1_in = nc.dram_tensor("mw1_in", [G, P, C * C], F16, kind="ExternalInput")
    abw0t_in = nc.dram_tensor("abw0t_in", [C, C], F32, kind="ExternalInput")
    abw0_in = nc.dram_tensor("abw0_in", [P, C * C], F32, kind="ExternalInput")
    abw1_in = nc.dram_tensor("abw1_in", [P, C * C], F32, kind="ExternalInput")
    wsw0_in = nc.dram_tensor("wsw0_in", [P, 3 * C], F32, kind="ExternalInput")
    wsw1_in = nc.dram_tensor("wsw1_in", [P, 3 * C], F32, kind="ExternalInput")
    cvec_in = nc.dram_tensor("cvec_in", [P, C], F32, kind="ExternalInput")
    predw_in = nc.dram_tensor("predw_in", [P, G * 2], F32, kind="ExternalInput")
    predb_in = nc.dram_tensor("predb_in", [1, 2], F32, kind="ExternalInput")
    ones_in = nc.dram_tensor("ones_in", [P, 1], F32, kind="ExternalInput")
    out_t = nc.dram_tensor("out_part", [1, 2], F32, kind="ExternalOutput")

    groups = [list(range(NCORES))]

    GN = 4  # node-update batch width (groups)
    assert G % GN == 0
    NB = G // GN

    def _batch_ready(bi):
        done = set()
        for pi, (a, b) in enumerate(pairs):
            done.add(a)
            done.add(b)
            if all(g in done for g in range(bi * GN, (bi + 1) * GN)):
                return pi
        return len(pairs) - 1

    b_ready = {bi: _batch_ready(bi) for bi in range(NB)}
    # AG1 in 2 chunks: every batch that finishes before the final pair rides
    # one early collective (hidden under layer-0 edge work); only the
    # last-finishing batches ride the small exposed final collective.
    # (each collective op costs ~17us of CC time regardless of payload)
    last_pi = len(pairs) - 1
    late = [bi for bi in range(NB) if b_ready[bi] >= last_pi]
    early = sorted((bi for bi in range(NB) if bi not in late),
                   key=lambda bi: (b_ready[bi], bi))
    ag_chunks = [c for c in (early, late) if c]
    batch_chunk = {}
    batch_coff = {}
    for k, bs in enumerate(ag_chunks):
        for i, bi in enumerate(bs):
            batch_chunk[bi] = k
            batch_coff[bi] = i
    chunk_ready = {k: max(b_ready[bi] for bi in bs) for k, bs in enumerate(ag_chunks)}

    with tile.TileContext(nc) as tc:
        with (
            tc.tile_pool(name="const", bufs=1) as cpool,
            tc.tile_pool(name="resident", bufs=1) as rpool,
            tc.tile_pool(name="gath", bufs=8) as gpool,
            tc.tile_pool(name="work", bufs=2) as wpool,
            tc.tile_pool(name="sinp", bufs=2) as snpool,
            tc.tile_pool(name="small", bufs=3) as spool,
            tc.tile_pool(name="wstream", bufs=2) as wspool,
            tc.tile_pool(name="hm0", bufs=2) as hpool,
            tc.tile_pool(name="psum", bufs=2, space="PSUM") as ppool,
            tc.tile_pool(name="psumf", bufs=1, space="PSUM") as pfpool,
            tc.tile_pool(name="dram", bufs=1, space="DRAM") as dpool,
        ):
            nc.gpsimd.load_library(library_config.mlp)

            # warm the Q7 gather path (iram fetch + ring setup) with a tiny
            # dummy gather so the first real call doesn't pay the cold cost
            wu_tab = dpool.tile([1, ROW], F32, name="wu_tab")
            wu_idx = cpool.tile([P, 8], I16)
            wu_out = cpool.tile([P, C], F32)
            nc.vector.memset(wu_idx[:], 0)
            _eng = nc.gpsimd
            _eng.add_instruction(
                mybir.InstDMAGatherAnt(
                    name=_eng.bass.get_next_instruction_name(),
                    ins=[
                        *_eng.lower_ap_dma(wu_tab[:, 0:C], for_custom_bir_dma=True),
                        _eng.lower_ap(wu_idx[:]),
                        _eng.lower_val_access(_eng.to_reg(P)),
                    ],
                    outs=[_eng.lower_ap(wu_out[:].rearrange("p (d e) -> p d e", e=C))],
                    transpose=False,
                    num_idxs=P,
                    elem_size=C,
                    stride_bytes_256=(ROW * 4) // 256,
                    gen_mode=0,
                    single_packet=False,
                    queue_num=0,
                )
            )

            # ---- constants / resident data ----
            cvec_sb = cpool.tile([P, C], F32)
            abw0t_sb = cpool.tile([C, C], F32)
            abw1_sb = cpool.tile([P, C * C], F32)
            wsw0_sb = cpool.tile([P, 3 * C], F32)
            wsw1_sb = cpool.tile([P, 3 * C], F32)
            ones_sb = cpool.tile([P, 1], F32)
            predb_sb = cpool.tile([1, 2], F32)

            idx_sb = rpool.tile([P, IW], I16)
            a0_sb = rpool.tile([P, G * C], F32)
            posd_sb = rpool.tile([P, G * 3], F32)
            possrc_sb = rpool.tile([P, SD * 3], F32)
            h0_sb = rpool.tile([P, G * C], F32)
            predw_sb = rpool.tile([P, G * 2], F32)
            remb_sb = rpool.tile([P, RW], F16)
            h1_sb = rpool.tile([P, G * C], F32)
            acc_sb = rpool.tile([P, 2], F32)

            nc.scalar.dma_start(abw0t_sb[:], abw0t_in[:])
            nc.gpsimd.dma_start(idx_sb[:], idx_in[:])
            nc.sync.dma_start(possrc_sb[:], possrc_in[:])
            nc.sync.dma_start(posd_sb[:], posd_in[:])
            nc.sync.dma_start(h0_sb[:], h0_in[:])
            nc.scalar.dma_start(cvec_sb[:], cvec_in[:])
            nc.scalar.dma_start(abw1_sb[:], abw1_in[:])
            nc.scalar.dma_start(wsw0_sb[:], wsw0_in[:])
            nc.scalar.dma_start(wsw1_sb[:], wsw1_in[:])
            nc.scalar.dma_start(ones_sb[:], ones_in[:])
            nc.scalar.dma_start(predb_sb[:], predb_in[:])
            nc.scalar.dma_start(predw_sb[:], predw_in[:])
            nc.vector.memset(acc_sb[:], 0.0)
            # fold remb's sqrt(2) into the layer-0/1 atomic-basis weights
            nc.vector.tensor_scalar_mul(
                out=abw0t_sb[:], in0=abw0t_sb[:], scalar1=float(np.sqrt(2.0))
            )
            nc.vector.tensor_scalar_mul(
                out=abw1_sb[:], in0=abw1_sb[:], scalar1=float(np.sqrt(2.0))
            )

            # ---- DRAM: wide gather tables (rows: p*GT + gg) + AG1 bounces
            wtab0 = dpool.tile([ntab, ROW], F32)
            wtab1 = dpool.tile([ntab, ROW], F32)
            ag_ins = [
                dpool.tile([len(bs) * GN * P, C], F32, name=f"ag1i_{k}")
                for k, bs in enumerate(ag_chunks)
            ]
            ag_outs = [
                dpool.tile([NCORES * len(bs) * GN * P, C], F32, name=f"ag1o_{k}")
                for k, bs in enumerate(ag_chunks)
            ]

            # ---- layer-0 full hm table, replicated: PE (112 groups) and
            # DVE (48 groups) build it concurrently ----
            widen_engines = [nc.sync, nc.scalar, nc.gpsimd]
            h0tail_sb = rpool.tile([P, DVG * C], F32)
            nc.scalar.dma_start(h0tail_sb[:], h0tail_in[:])
            abw0_sb = cpool.tile([P, C * C], F32)
            nc.scalar.dma_start(abw0_sb[:], abw0_in[:])
            nc.vector.tensor_scalar_mul(
                out=abw0_sb[:], in0=abw0_sb[:], scalar1=float(np.sqrt(2.0))
            )
            for sb in range(NSLAB):
                h0t_t = hpool.tile([C, SLAB * P], F32, name=f"h0t_{sb}", tag="h0t")
                nc.sync.dma_start(
                    h0t_t[:], h0t_in[:, sb * SLAB * P : (sb + 1) * SLAB * P]
                )
                ps = ppool.tile([P, SLAB * C], F32, name=f"ps_{sb}", tag="ps")
                for gl in range(SLAB):
                    nc.tensor.matmul(
                        out=ps[:, gl * C : (gl + 1) * C],
                        lhsT=h0t_t[:, gl * P : (gl + 1) * P],
                        rhs=abw0t_sb[:],
                        start=True,
                        stop=True,
                    )
                hsl = hpool.tile([P, SLAB * ROW], F32, name=f"hsl_{sb}", tag="hsl")
                nc.vector.tensor_copy(
                    out=hsl[:].rearrange("p (g w) -> p g w", w=ROW)[:, :, 0:C],
                    in_=ps[:].rearrange("p (g c) -> p g c", c=C),
                )
                weng = widen_engines[sb % 3]
                weng.dma_start(
                    wtab0[:, :]
                    .rearrange("(p q) w -> p q w", q=GT)[
                        :, sb * SLAB : (sb + 1) * SLAB, :
                    ],
                    hsl[:].rearrange("p (g w) -> p g w", w=ROW),
                )

            for bb in range(DVG // GN0):
                gg0 = PEG + bb * GN0
                h8 = h0tail_sb[:, bb * GN0 * C : (bb + 1) * GN0 * C]
                t = hpool.tile([P, GN0 * C * C], F32, name=f"hdT_{bb}", tag="hdT")
                nc.vector.tensor_mul(
                    out=t[:].rearrange("p (g j k) -> p g j k", j=C, k=C),
                    in0=abw0_sb[:]
                    .rearrange("p (j k) -> p j k", k=C)
                    .to_broadcast([P, C, C, GN0])
                    .rearrange("p j k g -> p g j k"),
                    in1=h8.rearrange("p (g k) -> p g k", k=C)
                    .to_broadcast([P, GN0, C, C])
                    .rearrange("p g k j -> p g j k"),
                )
                hd = hpool.tile([P, GN0 * ROW], F32, name=f"hd_{bb}", tag="hd")
                nc.vector.reduce_sum(
                    out=hd[:].rearrange("p (g w) -> p g w", w=ROW)[:, :, 0:C],
                    in_=t[:].rearrange("p (g j k) -> p g j k", j=C, k=C),
                    axis=mybir.AxisListType.X,
                )
                weng = widen_engines[bb % 3]
                weng.dma_start(
                    wtab0[:, :]
                    .rearrange("(p q) w -> p q w", q=GT)[:, gg0 : gg0 + GN0, :],
                    hd[:].rearrange("p (g w) -> p g w", w=ROW),
                )

            # ---- remb/sin chain for every pair (no gather dependency) ----
            DPmax = max(Dg[ga] + Dg[gb] for ga, gb in pairs)

            def remb_pair(ga, gb):
                Da, Db = Dg[ga], Dg[gb]
                DP = Da + Db
                remb_g = remb_sb[:, off_s[ga] * C : (off_s[ga] + DP) * C]
                ps_ = possrc_sb[:, off_s[ga] * 3 : (off_s[ga] + DP) * 3]
                rel_t = spool.tile([P, DPmax * 3], F32, name=f"rel_{ga}", tag="rel")
                rel = rel_t[:, 0 : DP * 3]
                for gg, doff in ((ga, 0), (gb, Da)):
                    D = Dg[gg]
                    nc.vector.tensor_sub(
                        out=rel_t[:, doff * 3 : (doff + D) * 3].rearrange(
                            "p (d x) -> p d x", x=3
                        ),
                        in0=ps_.rearrange("p (d x) -> p d x", x=3)[
                            :, doff : doff + D, :
                        ],
                        in1=posd_sb[:, gg * 3 : (gg + 1) * 3]
                        .to_broadcast([P, 3, D])
                        .rearrange("p x d -> p d x"),
                    )
                nc.vector.tensor_mul(out=rel, in0=rel, in1=rel)
                dd_t = spool.tile([P, DPmax], F32, name=f"dd_{ga}", tag="dd")
                dd = dd_t[:, 0:DP]
                nc.vector.reduce_sum(
                    out=dd,
                    in_=rel.rearrange("p (d x) -> p d x", x=3),
                    axis=mybir.AxisListType.X,
                )
                dv_t = spool.tile([P, DPmax], F32, name=f"dv_{ga}", tag="dv")
                dv = dv_t[:, 0:DP]
                nc.scalar.sqrt(dv, dd)
                inv_t = spool.tile([P, DPmax], F32, name=f"inv_{ga}", tag="inv")
                inv = inv_t[:, 0:DP]
                nc.vector.reciprocal(inv, dv)
                sin_t = snpool.tile([P, DPmax * C], F32, name=f"sinin_{ga}", tag="sin")
                sinin = sin_t[:, 0 : DP * C]
                nc.vector.tensor_mul(
                    out=sinin.rearrange("p (d c) -> p d c", c=C),
                    in0=dv.to_broadcast([P, DP, C]),
                    in1=cvec_sb[:].to_broadcast([P, C, DP]).rearrange("p c d -> p d c"),
                )
                MAGIC = 12582912.0  # 1.5 * 2**23
                kr_t = snpool.tile([P, DPmax * C], F32, name=f"kr_{ga}", tag="kr")
                kr = kr_t[:, 0 : DP * C]
                # kr = round(sinin) via fp32 magic add; y = round - sinin =
                # -frac (fused); sin(2*pi*frac) = Sin(-2*pi * y)
                nc.vector.tensor_scalar_add(out=kr, in0=sinin, scalar1=MAGIC)
                nc.vector.scalar_tensor_tensor(
                    out=kr,
                    in0=kr,
                    scalar=MAGIC,
                    in1=sinin,
                    op0=mybir.AluOpType.subtract,
                    op1=mybir.AluOpType.subtract,
                )
                sino = sinin
                nc.scalar.activation(
                    sino,
                    kr,
                    mybir.ActivationFunctionType.Sin,
                    scale=float(-2.0 * np.pi),
                )
                nc.vector.tensor_mul(
                    out=remb_g.rearrange("p (d c) -> p d c", c=C),
                    in0=sino.rearrange("p (d c) -> p d c", c=C),
                    in1=inv.to_broadcast([P, DP, C]),
                )

            for ga, gb in pairs:
                remb_pair(ga, gb)

            # ---- edge processing (both layers) ----
            def gather_small(out_ap, in_ap, idxs_ap, num_idxs, queue_num):
                """InstDMAGatherAnt with a 64B payload per index (the %256
                elem_size assert in bass.dma_gather is a transpose-mode
                restriction; the non-transpose Q7 generator handles any
                packet length — only the row STRIDE must be 256B-aligned)."""
                eng = nc.gpsimd
                _in = eng.lower_ap_dma(in_ap, for_custom_bir_dma=True)
                _idx = eng.lower_ap(idxs_ap)
                _out = eng.lower_ap(out_ap)
                return eng.add_instruction(
                    mybir.InstDMAGatherAnt(
                        name=eng.bass.get_next_instruction_name(),
                        ins=[*_in, _idx, eng.lower_val_access(eng.to_reg(num_idxs))],
                        outs=[_out],
                        transpose=False,
                        num_idxs=num_idxs,
                        elem_size=C,
                        stride_bytes_256=(ROW * 4) // 256,
                        gen_mode=0,
                        single_packet=False,
                        queue_num=queue_num,
                    )
                )

            def edge_layer_pair(pi, ga, gb, wtab):
                Da, Db = Dg[ga], Dg[gb]
                DP = Da + Db
                remb_g = remb_sb[:, off_s[ga] * C : (off_s[ga] + DP) * C]
                gt = gpool.tile([P, DPmax * C], F32, name=f"gt_{ga}", tag="gt")
                gv3 = gt[:].rearrange("p (d e) -> p d e", e=C)
                for ci, (doff, dq) in enumerate(plan[pi]):
                    io = off_pc[pi][ci]
                    gather_small(
                        out_ap=gv3[:, doff : doff + dq, :],
                        in_ap=wtab[:, 0:C],
                        idxs_ap=idx_sb[:, io : io + 8 * dq],
                        num_idxs=P * dq,
                        queue_num=next_q(),
                    )
                he = gv3[:, 0:DP, :]
                nc.vector.tensor_mul(
                    out=he,
                    in0=remb_g.rearrange("p (d c) -> p d c", c=C),
                    in1=he,
                )
                for gg, doff in ((ga, 0), (gb, Da)):
                    D = Dg[gg]
                    nc.vector.reduce_sum(
                        out=a0_sb[:, gg * C : (gg + 1) * C],
                        in_=gt[:].rearrange("p (d e) -> p e d", e=C)[
                            :, :, doff : doff + D
                        ],
                        axis=mybir.AxisListType.X,
                    )

            def wsw_b(wsw_sb, k):
                return (
                    wsw_sb[:, k * C : (k + 1) * C]
                    .to_broadcast([P, C, GN])
                    .rearrange("p c g -> p g c")
                )

            def node_update_batch(b, wsw_sb, cw_in_t, mw_in_t, h_src_sb, h_dst_ap):
                s = a0_sb[:, b * C : (b + GN) * C]
                sv = s.rearrange("p (g c) -> p g c", c=C)
                tp = spool.tile([P, GN * C], F32, name=f"tp_{b}", tag="tp")
                tv = tp[:].rearrange("p (g c) -> p g c", c=C)
                nc.vector.tensor_mul(out=tv, in0=sv, in1=wsw_b(wsw_sb, 2))
                nc.vector.tensor_add(out=tv, in0=tv, in1=wsw_b(wsw_sb, 1))
                nc.vector.tensor_mul(out=tp[:], in0=tp[:], in1=s)
                nc.vector.tensor_add(out=tv, in0=tv, in1=wsw_b(wsw_sb, 0))
                msgs = spool.tile([P, GN * C], F32, name=f"msgs_{b}", tag="msgs")
                nc.vector.tensor_mul(out=msgs[:], in0=tp[:], in1=s)

                cw_t = wspool.tile([P, GN * C * C], F16, name=f"cw_{b}", tag="cw")
                mw_t = wspool.tile([P, GN * C * C], F16, name=f"mw_{b}", tag="mw")
                nc.scalar.dma_start(
                    cw_t[:].rearrange("p (g w) -> p g w", g=GN),
                    cw_in_t[:].rearrange("g p w -> p g w")[:, b : b + GN, :],
                )
                nc.scalar.dma_start(
                    mw_t[:].rearrange("p (g w) -> p g w", g=GN),
                    mw_in_t[:].rearrange("g p w -> p g w")[:, b : b + GN, :],
                )
                h4 = h_src_sb[:, b * C : (b + GN) * C]
                t1 = wpool.tile([P, GN * C * C], F32, name=f"t1_{b}", tag="t1")
                nc.vector.tensor_mul(
                    out=t1[:].rearrange("p (g j k) -> p g j k", j=C, k=C),
                    in0=cw_t[:].rearrange("p (g j k) -> p g j k", j=C, k=C),
                    in1=h4.rearrange("p (g k) -> p g k", k=C)
                    .to_broadcast([P, GN, C, C])
                    .rearrange("p g k j -> p g j k"),
                )
                t2 = wpool.tile([P, GN * C * C], F32, name=f"t2_{b}", tag="t2")
                nc.vector.tensor_mul(
                    out=t2[:].rearrange("p (g j k) -> p g j k", j=C, k=C),
                    in0=mw_t[:].rearrange("p (g j k) -> p g j k", j=C, k=C),
                    in1=msgs[:].rearrange("p (g k) -> p g k", k=C)
                    .to_broadcast([P, GN, C, C])
                    .rearrange("p g k j -> p g j k"),
                )
                nc.vector.tensor_add(out=t1[:], in0=t1[:], in1=t2[:])
                nc.vector.reduce_sum(
                    out=h_dst_ap,
                    in_=t1[:].rearrange("p (g j k) -> p g j k", j=C, k=C),
                    axis=mybir.AxisListType.X,
                )

            def hm1_batch(b):
                """hm1 for groups [b, b+GN) -> 256B-padded AG1 chunk rows."""
                h4 = h1_sb[:, b * C : (b + GN) * C]
                t = wpool.tile([P, GN * C * C], F32, name=f"hmT_{b}", tag="t1")
                nc.vector.tensor_mul(
                    out=t[:].rearrange("p (g j k) -> p g j k", j=C, k=C),
                    in0=abw1_sb[:]
                    .rearrange("p (j k) -> p j k", k=C)
                    .to_broadcast([P, C, C, GN])
                    .rearrange("p j k g -> p g j k"),
                    in1=h4.rearrange("p (g k) -> p g k", k=C)
                    .to_broadcast([P, GN, C, C])
                    .rearrange("p g k j -> p g j k"),
                )
                hm = spool.tile([P, GN * C], F32, name=f"hm_{b}", tag="hm")
                nc.vector.reduce_sum(
                    out=hm[:],
                    in_=t[:].rearrange("p (g j k) -> p g j k", j=C, k=C),
                    axis=mybir.AxisListType.X,
                )
                bi = b // GN
                k = batch_chunk[bi]
                off = batch_coff[bi]
                qn = len(ag_chunks[k]) * GN
                dst = ag_ins[k][:].rearrange("(p q) c -> p q c", q=qn)[
                    :, off * GN : (off + 1) * GN, :
                ]
                nc.sync.dma_start(dst, hm[:].rearrange("p (g c) -> p g c", c=C))

            def ag1_chunk(k):
                nc.gpsimd.collective_compute(
                    "AllGather",
                    mybir.AluOpType.bypass,
                    replica_groups=groups,
                    ins=[ag_ins[k][:].opt()],
                    outs=[ag_outs[k][:].opt()],
                )

            def widen1_chunk(k, engines):
                """ag_out rows (n, p, q) compact -> wtab1 rows p*GT + n*G + g
                (64B payload per 256B-strided row), per batch per core."""
                bs = ag_chunks[k]
                qn = len(bs) * GN
                src = ag_outs[k][:].rearrange("(n p q) c -> n p q c", n=NCORES, q=qn)
                dstv = wtab1[:, 0:C].rearrange("(p n g) w -> n p g w", n=NCORES, g=G)
                e = 0
                for i, bi in enumerate(bs):
                    g0 = bi * GN
                    for n in range(NCORES):
                        engines[e % len(engines)].dma_start(
                            dstv[n, :, g0 : g0 + GN, :],
                            src[n, :, i * GN : (i + 1) * GN, :],
                        )
                        e += 1

            def batches_ready_after(pi):
                done = set()
                for q in range(pi + 1):
                    done.add(pairs[q][0])
                    done.add(pairs[q][1])
                return [
                    b for b in range(0, G, GN)
                    if all(g in done for g in range(b, b + GN))
                ]

            # ---- layer 0 ----
            CC_LAG = 0
            emitted = set()
            emitted_cc = set()
            emitted_w = set()
            for pi, (ga, gb) in enumerate(pairs):
                edge_layer_pair(pi, ga, gb, wtab0)
                for b in batches_ready_after(pi):
                    if b in emitted:
                        continue
                    emitted.add(b)
                    node_update_batch(
                        b, wsw0_sb, cw0_in, mw0_in, h0_sb,
                        h1_sb[:, b * C : (b + GN) * C],
                    )
                    hm1_batch(b)
                for k, rp in chunk_ready.items():
                    if k not in emitted_cc and pi >= rp + CC_LAG:
                        emitted_cc.add(k)
                        ag1_chunk(k)
                    if k in emitted_cc and k not in emitted_w and pi >= rp + CC_LAG + 1:
                        emitted_w.add(k)
                        widen1_chunk(k, [nc.sync, nc.scalar])
            for k in range(len(ag_chunks)):
                if k not in emitted_cc:
                    ag1_chunk(k)
            for k in range(len(ag_chunks)):
                if k not in emitted_w:
                    # at the layer boundary the Pool queue is free too
                    widen1_chunk(k, [nc.sync, nc.scalar, nc.gpsimd])

            # ---- layer 1 + head ----
            def head_batch(b):
                hf = spool.tile([P, GN * C], F32, name=f"hf_{b}", tag="hf")
                node_update_batch(b, wsw1_sb, cw1_in, mw1_in, h1_sb, hf[:])
                rs = spool.tile([P, GN], F32, name=f"rs_{b}", tag="rs")
                nc.vector.reduce_sum(
                    out=rs[:],
                    in_=hf[:].rearrange("p (g c) -> p g c", c=C),
                    axis=mybir.AxisListType.X,
                )
                ctr = spool.tile([P, GN * 2], F32, name=f"ctr_{b}", tag="ctr")
                nc.vector.tensor_mul(
                    out=ctr[:].rearrange("p (g t) -> p g t", t=2),
                    in0=predw_sb[:, b * 2 : (b + GN) * 2].rearrange(
                        "p (g t) -> p g t", t=2
                    ),
                    in1=rs[:].to_broadcast([P, GN, 2]),
                )
                cts = spool.tile([P, 2], F32, name=f"cts_{b}", tag="cts")
                nc.vector.reduce_sum(
                    out=cts[:],
                    in_=ctr[:].rearrange("p (g t) -> p t g", t=2),
                    axis=mybir.AxisListType.X,
                )
                nc.vector.tensor_add(out=acc_sb[:], in0=acc_sb[:], in1=cts[:])

            emitted = set()
            for pi, (ga, gb) in enumerate(pairs):
                edge_layer_pair(pi, ga, gb, wtab1)
                for b in batches_ready_after(pi):
                    if b in emitted:
                        continue
                    emitted.add(b)
                    head_batch(b)

            fin_ps = pfpool.tile([1, 2], F32)
            nc.tensor.matmul(
                out=fin_ps[:], lhsT=ones_sb[:], rhs=acc_sb[:], start=True, stop=True
            )
            res = spool.tile([1, 2], F32)
            nc.vector.tensor_add(out=res[:], in0=fin_ps[:], in1=predb_sb[:])
            nc.sync.dma_start(out_t[:], res[:])

    nc.compile()
    return nc


def _wrap_idx16(v):
    """[128, D] int array -> dma_gather idx tile [128, 8*D] int16."""
    p128, d_pad = v.shape
    blk = v.T.reshape(d_pad, 8, 16).transpose(2, 0, 1).reshape(16, 8 * d_pad)
    return np.tile(blk, (8, 1)).astype(np.int16)


def _prep_inputs(pos, h0, ab_w, ws_w, cw, mw, pred_w, pred_b, edge_index):
    """Host-side sharding: degree-sort nodes per core, bucket edges by dst
    into per-group padded degree slots, slice per-node weights.  Data
    movement / index arithmetic only — all model arithmetic runs on
    device."""
    pos = np.asarray(pos, np.float32)
    h0 = np.asarray(h0, np.float32)
    ab_w = np.asarray(ab_w, np.float32)
    ws_w = np.asarray(ws_w, np.float32)
    cw = np.asarray(cw, np.float32)
    mw = np.asarray(mw, np.float32)
    pred_w = np.asarray(pred_w, np.float32)
    pred_b = np.asarray(pred_b, np.float32)
    ei = np.asarray(edge_index)
    src = ei[0].astype(np.int64)
    dst = ei[1].astype(np.int64)

    n_nodes = pos.shape[0]
    c_ch = h0.shape[1]
    assert n_nodes % NCORES == 0
    nc_nodes = n_nodes // NCORES
    g_groups = -(-nc_nodes // P)
    npad = g_groups * P
    GT = NCORES * g_groups
    assert P * GT <= 32767, "int16 gather indices"

    deg_all = np.bincount(dst, minlength=n_nodes)

    perms = []
    inv_slot = np.zeros(n_nodes, np.int64)
    for core in range(NCORES):
        lo = core * nc_nodes
        order = np.argsort(-deg_all[lo : lo + nc_nodes], kind="stable")
        perm = np.concatenate([order, np.arange(nc_nodes, npad)])
        perms.append(perm)
        inv_slot[lo + order] = np.arange(nc_nodes)

    deg_sorted = np.zeros((NCORES, npad), np.int64)
    for core in range(NCORES):
        lo = core * nc_nodes
        deg_sorted[core, :nc_nodes] = deg_all[lo + perms[core][:nc_nodes]]
    Dg = []
    for g in range(g_groups):
        dmax = int(deg_sorted[:, g * P : (g + 1) * P].max())
        Dg.append(max(1, dmax))
    Dg = tuple(Dg)

    pairs = _pair_order(g_groups)
    off_s = [0] * g_groups
    acc_s = 0
    for ga, gb in pairs:
        off_s[ga] = acc_s
        acc_s += Dg[ga]
        off_s[gb] = acc_s
        acc_s += Dg[gb]
    SD = acc_s

    def padded_row(node):
        # table row (partition-major): p*GT + core*G + g
        core = node // nc_nodes
        s = inv_slot[node]
        return (s % P) * GT + core * g_groups + s // P

    # full-table h0 in (gg, p) order; transposed head for the PE, row-major
    # tail for DVE
    PEG = 96
    h0tab = np.zeros((GT, P, c_ch), np.float32)
    for core in range(NCORES):
        lo = core * nc_nodes
        perm = perms[core]
        real = perm < nc_nodes
        hvals = np.zeros((npad, c_ch), np.float32)
        hvals[real] = h0[lo + perm[real], :, 0]
        h0tab[core * g_groups : (core + 1) * g_groups] = hvals.reshape(
            g_groups, P, c_ch
        )
    h0t = (
        h0tab[:PEG].reshape(PEG * P, c_ch).T.copy()
    )  # [C, PEG*P], col = gg*128+p
    h0tail = (
        h0tab[PEG:].transpose(1, 0, 2).reshape(P, (GT - PEG) * c_ch).copy()
    )  # [P, (gg-PEG)*C]

    in_maps = []
    for core in range(NCORES):
        lo, hi = core * nc_nodes, (core + 1) * nc_nodes
        sel = (dst >= lo) & (dst < hi)
        d_loc = inv_slot[dst[sel]]
        s_glb = src[sel]
        order = np.argsort(d_loc, kind="stable")
        d_sort = d_loc[order]
        s_sort = s_glb[order]
        starts = np.searchsorted(d_sort, np.arange(nc_nodes))
        rank = np.arange(len(d_sort)) - starts[d_sort]

        perm = perms[core]
        real = perm < nc_nodes

        posd = np.zeros((npad, 3), np.float32)
        posd[real] = pos[lo + perm[real]]
        posd[~real, 0] = 1e4 + core * npad + np.arange(
            npad - nc_nodes, dtype=np.float32
        )

        # dummy: a PAD row of the next core (its h0t cols are 0)
        sd = nc_nodes  # first pad slot
        dummy = (sd % P) * GT + ((core + 1) % NCORES) * g_groups + sd // P
        idxg = {}
        possrc = np.zeros((P, SD, 3), np.float32)
        for g in range(g_groups):
            D = Dg[g]
            idx = np.full((P, D), dummy, np.int64)
            srcm = np.full((P, D), -1, np.int64)
            in_g = (d_sort >= g * P) & (d_sort < (g + 1) * P)
            rg = d_sort[in_g] - g * P
            rk = rank[in_g]
            idx[rg, rk] = padded_row(s_sort[in_g])
            srcm[rg, rk] = s_sort[in_g]
            idxg[g] = idx
            ps = np.empty((P, D, 3), np.float32)
            valid = srcm >= 0
            ps[valid] = pos[srcm[valid]]
            padmask = ~valid
            pd, _ = np.nonzero(padmask)
            base = posd[g * P : (g + 1) * P]
            ps[padmask] = base[pd] + np.array([1.7, 0.0, 0.0], np.float32)
            possrc[:, off_s[g] : off_s[g] + D, :] = ps
        # pair-major idx layout, wrapped per merged-pair chunk
        plan = _chunk_plan(Dg, pairs)
        idx_cols = []
        for pi, (ga, gb) in enumerate(pairs):
            idxp = np.concatenate([idxg[ga], idxg[gb]], axis=1)
            for doff, dq in plan[pi]:
                idx_cols.append(_wrap_idx16(idxp[:, doff : doff + dq]))
        idx_w = np.concatenate(idx_cols, axis=1)

        h0loc = np.zeros((npad, c_ch), np.float32)
        h0loc[real] = h0[lo + perm[real], :, 0]

        def node_w(warr, layer):
            wloc = np.zeros((npad, c_ch * c_ch), np.float16)
            wloc[real] = (
                warr[layer, 0, lo + perm[real]]
                .reshape(-1, c_ch * c_ch)
                .astype(np.float16)
            )
            return wloc.reshape(g_groups, P, c_ch * c_ch)

        predw = np.zeros((npad, 2), np.float32)
        predw[real] = pred_w[:, lo + perm[real]].T

        rep = lambda v: np.broadcast_to(
            np.asarray(v, np.float32).reshape(1, -1), (P, np.asarray(v).size)
        ).copy()

        def part_major(a):
            K = a.shape[1]
            return (
                a.reshape(g_groups, P, K).transpose(1, 0, 2).reshape(P, g_groups * K)
            )

        in_maps.append(
            {
                "idx_in": idx_w,
                "posd_in": part_major(posd),
                "possrc_in": possrc.reshape(P, SD * 3),
                "h0t_in": h0t,
                "h0tail_in": h0tail,
                "h0_in": part_major(h0loc),
                "cw0_in": node_w(cw, 0),
                "mw0_in": node_w(mw, 0),
                "cw1_in": node_w(cw, 1),
                "mw1_in": node_w(mw, 1),
                "abw0t_in": ab_w[0, 0].T.copy(),
                "abw0_in": rep(ab_w[0, 0].ravel()),
                "abw1_in": rep(ab_w[1, 0].ravel()),
                "wsw0_in": rep(ws_w[0, 0].ravel()),
                "wsw1_in": rep(ws_w[1, 0].ravel()),
                "cvec_in": rep(np.arange(1, c_ch + 1, dtype=np.float32) / 2.0),
                "predw_in": part_major(predw),
                "predb_in": (pred_b if core == 0 else np.zeros(2)).reshape(1, 2)
                .astype(np.float32),
                "ones_in": np.ones((P, 1), np.float32),
            }
        )
    meta = dict(Dg=Dg, C=c_ch, npad=npad)
    return in_maps, meta


_NC_CACHE = {}


def kernel(**inputs) -> np.ndarray:
    in_maps, meta = _prep_inputs(**inputs)
    key = (meta["Dg"], meta["C"], meta["npad"])
    if key not in _NC_CACHE:
        _NC_CACHE[key] = _build_nc(**meta)
    nc = _NC_CACHE[key]
    res = run_bass_kernel_spmd(nc, in_maps, core_ids=list(range(NCORES)))
    parts = [r["out_part"] for r in res.results]
    return np.sum(parts, axis=0).astype(np.float32)
